# revision 1
# baseline (speedup 1.0000x reference)
"""Ragged -> padded batch scatter (BatchedSequences) on 8 TRN2 NeuronCores.

Reference semantics: rows of concatenated_sequences [T, F] are scattered into
a zero-padded output [B, max_sl, F] according to per-sequence lengths.

Strategy (pure data movement, memory-bound):
  - All sequence lengths are multiples of 64 rows, so all sequence
    boundaries align to 16-row "superchunks" (16*512 f32 = 32 KiB).
  - Shard sequences across 8 cores with a balanced pairing so every core
    moves the same number of rows -> a single uniform SPMD program.
  - Per core: stream contiguous 4 MiB groups HBM->SBUF with direct DMA
    (one superchunk per SBUF partition), then one indirect scatter per
    group writes each partition's superchunk to its destination offset in
    the padded per-core output, driven by a host-computed int32 index
    tensor ([128, 1] per group, one index per partition).
  - Padding stays zero because run_bass_kernel_spmd pre-zeroes / donates
    zero-filled ExternalOutput buffers.
  - Measured on TRN2: ~131-137 us NEFF exec (best 131.2), the SBUF-AXI roofline
    (2 x 25.4 MB per core through 16 ports @ 27.2 GB/s = ~118 us of DMA
    work) plus ~10 us kernel start/drain overhead.
"""

from contextlib import ExitStack

import numpy as np

import concourse.bass as bass
import concourse.mybir as mybir
from concourse.bass_utils import run_bass_kernel_spmd

B = 32
F = 512
MAX_SL = 4096
NCORES = 8
SEQ_PER_CORE = B // NCORES
CHUNK = 64                       # rows per length-granularity chunk
SUP_EL = 8192                    # f32 elements per superchunk = 32 KiB
SUP_ROWS = SUP_EL // F           # 16 rows per superchunk
OUT_CHUNKS = SEQ_PER_CORE * MAX_SL // CHUNK   # 256 data chunks per core

_NC_CACHE: dict[int, bass.Bass] = {}


NSLOTS = 3  # staging buffers


def _group_plan(n_rows: int):
    """Split n_rows into groups of (rows, extent_rows). Each group occupies
    rows/extent <= 128 partitions. Workhorse groups are 2048 rows at
    16-row extents (32 KB/partition, one superchunk per partition). n_rows
    must be a multiple of SUP_ROWS."""
    plan = []
    rem = n_rows
    while rem >= 2048:
        plan.append((2048, 16))
        rem -= 2048
    if rem:
        e = 8 if rem // 8 <= 128 else 16
        plan.append((rem, e))
    # small groups FIRST: shorter ramp, and the big trailing groups keep the
    # engines covered while the final scatter's receipt+gen latency plays out
    plan.sort(key=lambda g: g[0])
    return plan


def _build_nc(n_sups: int) -> bass.Bass:
    """Uniform per-core program: scatter superchunks of x into superchunks of
    y selected by dst. y has one extra trash chunk for padded (unused) source
    superchunks.

    HW indirect-DMA contract (probed): offsets live one-per-partition
    ([P, 1] int32); for index p the DMA moves in_'s partition-p free extent
    (E elements) to out.flat[idx[p]*coef : +E], where coef is the product of
    the out-AP dims after the indirect axis. We keep E == coef per group.

    Raw Bass (no Tile): DMA queue instructions only support a single attached
    sync-wait, so all waits are standalone sequencer instructions. DMAs signal
    per-slot semaphores reused cyclically with rising thresholds. Scatters are
    not ordered among themselves (destination superchunks are disjoint by
    construction)."""
    nc = bass.Bass()
    n_rows = n_sups * SUP_ROWS
    x = nc.declare_dram_parameter("x", [n_rows, F], mybir.dt.float32, isOutput=False)
    plan = _group_plan(n_rows)
    ng = len(plan)

    dst = nc.declare_dram_parameter("dst", [128, ng], mybir.dt.int32, isOutput=False)
    y = nc.declare_dram_parameter(
        "y", [(OUT_CHUNKS + 1) * CHUNK, F], mybir.dt.float32, isOutput=True
    )

    with ExitStack() as ctx:
        slot_el = SUP_EL  # slot stride: one 32 KB superchunk per partition
        stage = ctx.enter_context(
            nc.sbuf_tensor([128, NSLOTS * slot_el], mybir.dt.float32)
        )
        dst_t = ctx.enter_context(nc.sbuf_tensor([128, ng], mybir.dt.int32))
        sem_dst = ctx.enter_context(nc.semaphore("sem_dst"))
        # per-slot semaphores, reused cyclically with rising thresholds (slot
        # reuse already serializes per-slot, so occurrence counts are
        # unambiguous); fewer sems -> shorter framework preamble
        sem_load = [ctx.enter_context(nc.semaphore(f"sem_load{s}")) for s in range(NSLOTS)]
        sem_scat = [ctx.enter_context(nc.semaphore(f"sem_scat{s}")) for s in range(NSLOTS)]
        block = ctx.enter_context(nc.Block())

        @block.scalar
        def _(scalar):
            # tiny index-table load on the second HWDGE ring, overlapping the
            # first data load on sync
            scalar.dma_start(out=dst_t[:, :], in_=dst[:, :]).then_inc(sem_dst, 16)

        @block.sync
        def _(sync):
            r0 = 0
            for g, (rows, ext) in enumerate(plan):
                parts = rows // ext
                slot = g % NSLOTS
                if g >= NSLOTS:
                    sync.wait_ge(sem_scat[slot], 16 * (g // NSLOTS))
                xin = x[r0 : r0 + rows, :].rearrange("(p q) f -> p (q f)", p=parts)
                sync.dma_start(
                    out=stage[:parts, slot * slot_el : slot * slot_el + ext * F],
                    in_=xin,
                ).then_inc(sem_load[slot], 16)
                r0 += rows

        @block.gpsimd
        def _(gp):
            slot_el = SUP_EL
            gp.wait_ge(sem_dst, 16)
            for g, (rows, ext) in enumerate(plan):
                parts = rows // ext
                slot = g % NSLOTS
                yv = y.rearrange("(n e) f -> n (e f)", e=ext)
                gp.wait_ge(sem_load[slot], 16 * (g // NSLOTS + 1))
                gp.indirect_dma_start(
                    out=yv[:, :],
                    out_offset=bass.IndirectOffsetOnAxis(
                        ap=dst_t[:parts, g : g + 1], axis=0
                    ),
                    in_=stage[:parts, slot * slot_el : slot * slot_el + ext * F],
                    in_offset=None,
                ).then_inc(sem_scat[slot], 16)
            for s in range(NSLOTS):
                uses = (ng - s + NSLOTS - 1) // NSLOTS
                gp.wait_ge(sem_scat[s], 16 * uses)
    return nc


def _plan(L: np.ndarray):
    """Assign SEQ_PER_CORE sequences to each core, balanced.

    Returns (groups, n_chunks) where groups[k] is the list of sequence ids on
    core k and n_chunks is the max chunk count across cores (cores with fewer
    chunks pad their dst with the trash chunk)."""
    assert len(L) == B
    # Pairing (i, B-1-i) balances linearly-decaying lengths exactly; fall back
    # to a greedy LPT assignment for arbitrary lengths.
    pair_groups = [
        [k, B - 1 - k, k + NCORES, B - 1 - k - NCORES] for k in range(NCORES)
    ]
    totals = [sum(int(L[s]) for s in g) for g in pair_groups]
    if max(totals) - min(totals) <= 2 * CHUNK:
        groups = pair_groups
    else:
        order = np.argsort(-L)
        groups = [[] for _ in range(NCORES)]
        gtot = [0] * NCORES
        for s in order:
            k = min(
                (k for k in range(NCORES) if len(groups[k]) < SEQ_PER_CORE),
                key=lambda k: gtot[k],
            )
            groups[k].append(int(s))
            gtot[k] += int(L[s])
    n_chunks = max(sum(int(L[s]) for s in g) for g in groups) // CHUNK
    return groups, n_chunks


def _host_fallback(S, L, max_sl):
    out = np.zeros((len(L), max_sl, S.shape[1]), dtype=S.dtype)
    off = 0
    for b, ln in enumerate(L):
        out[b, :ln] = S[off : off + ln]
        off += ln
    return out


def _prepare(S, L):
    """Host planning: returns (nc, in_maps, groups)."""
    offsets = np.zeros(B + 1, dtype=np.int64)
    np.cumsum(L, out=offsets[1:])

    groups, n_chunks = _plan(L)
    n_sups = n_chunks * CHUNK // SUP_ROWS
    n_rows = n_sups * SUP_ROWS
    plan = _group_plan(n_rows)
    ng = len(plan)
    trash_row = OUT_CHUNKS * CHUNK  # first row of the trash chunk

    in_maps = []
    for k in range(NCORES):
        xs = []
        for j, s in enumerate(groups[k]):
            ln = int(L[s])
            xs.append(S[offsets[s] : offsets[s] + ln])
        rows = sum(x.shape[0] for x in xs)
        pad_rows = n_rows - rows
        if pad_rows:
            xs.append(np.zeros((pad_rows, F), dtype=np.float32))
        x_k = np.concatenate(xs, axis=0)

        # destination out-row for every source row (pads -> trash chunk)
        dest_row = np.full(n_rows, trash_row, dtype=np.int64)
        pos = 0
        for j, s in enumerate(groups[k]):
            ln = int(L[s])
            dest_row[pos : pos + ln] = j * MAX_SL + np.arange(ln)
            pos += ln

        # dst layout [128, ng]: column g holds group g's per-partition indices
        # in units of that group's extent
        dst_k = np.zeros((128, ng), dtype=np.int32)
        r0 = 0
        for g, (grows, ext) in enumerate(plan):
            parts = grows // ext
            src = r0 + np.arange(parts) * ext
            dst_k[:parts, g] = dest_row[src] // ext
            r0 += grows
        in_maps.append({"x": x_k, "dst": np.ascontiguousarray(dst_k)})

    if n_sups not in _NC_CACHE:
        _NC_CACHE[n_sups] = _build_nc(n_sups)
    return _NC_CACHE[n_sups], in_maps, groups


def _assemble(results, groups):
    out = np.empty((B, MAX_SL, F), dtype=np.float32)
    for k in range(NCORES):
        yk = np.asarray(results[k]["y"])[: SEQ_PER_CORE * MAX_SL].reshape(
            SEQ_PER_CORE, MAX_SL, F
        )
        for j, s in enumerate(groups[k]):
            out[s] = yk[j]
    return out


def kernel(concatenated_sequences, sequence_lengths, max_sl):
    S = np.ascontiguousarray(np.asarray(concatenated_sequences, dtype=np.float32))
    L = np.asarray(sequence_lengths).reshape(-1).astype(np.int64)
    max_sl = int(np.asarray(max_sl))

    if (
        max_sl != MAX_SL
        or len(L) != B
        or S.shape[1] != F
        or int(L.sum()) != S.shape[0]
        or np.any(L % CHUNK)
        or np.any(L < 0)
        or np.any(L > max_sl)
    ):
        return _host_fallback(S, L, max_sl)

    nc, in_maps, groups = _prepare(S, L)
    res = run_bass_kernel_spmd(nc, in_maps, list(range(NCORES))).results
    return _assemble(res, groups)



# revision 2
# speedup vs baseline: 1.7221x; 1.7221x over previous
"""Ragged -> padded batch scatter (BatchedSequences) on 8 TRN2 NeuronCores.

Reference semantics: rows of concatenated_sequences [T, F] are scattered into
a zero-padded output [B, max_sl, F] according to per-sequence lengths.

Strategy (pure data movement, memory-bound):
  - Shard 4 sequences per core with the slot assignment
    groups[k] = [k, 15-k, 16+k, 31-k]; every core then owns exactly
    T/8 = 12416 rows (lengths decay linearly, slots pair them off).
  - Slot j's length varies per core, but its minimum over cores is a
    static "base" size (3648/3136/2624/2112 rows = 11520 of 12416 rows).
    Base pieces are copied with 4 big *direct DRAM->DRAM* DMAs whose
    sizes/offsets are identical on all 8 cores (pure SPMD), split over
    the two HWDGE rings (sync + scalar). No SBUF transit: each byte
    crosses the SDMA engines once instead of twice.
  - The ragged remainder (896 rows/core, 8-row granularity) goes through
    SBUF: one staged load + one indirect scatter driven by a
    host-computed [112, 1] int32 table of 8-row destination slots.
  - Host pre-arranges each core's x so all source offsets are static:
    [base_0 | base_1 | base_2 | base_3 | tail_0 | tail_1 | tail_2 | tail_3].
  - Padding stays zero because run_bass_kernel_spmd pre-zeroes / donates
    zero-filled ExternalOutput buffers.
"""

from contextlib import ExitStack

import numpy as np

import concourse.bass as bass
import concourse.mybir as mybir
from concourse.bass_utils import run_bass_kernel_spmd

B = 32
F = 512
MAX_SL = 4096
NCORES = 8
SEQ_PER_CORE = B // NCORES
RU = 8                          # remainder granularity: 8 rows = 16 KiB

_NC_CACHE: dict[tuple, bass.Bass] = {}


def _build_nc(bases: tuple[int, ...], rem_rows: int) -> bass.Bass:
    """Uniform per-core program.

    x [sum(bases)+rem_rows, F]: 4 base pieces at static offsets, then the
    ragged tails. y [SEQ_PER_CORE*MAX_SL, F]: slot j's region starts at
    j*MAX_SL. Base piece j: x[sb_j : sb_j+bases_j] -> y[j*MAX_SL : +bases_j]
    as direct DRAM->DRAM DMA (2 on the sync HWDGE ring, 2 on scalar's).
    Remainder: x[rem0 : rem0+rem_rows] staged to SBUF [P, RU*F], then one
    indirect scatter writes partition p's 8-row unit to y rows
    dst[p]*RU : +RU (dst is a per-core int32 table).
    """
    nc = bass.Bass()
    n_base = sum(bases)
    n_rows = n_base + rem_rows
    n_parts = rem_rows // RU
    x = nc.declare_dram_parameter("x", [n_rows, F], mybir.dt.float32, isOutput=False)
    dst = nc.declare_dram_parameter("dst", [n_parts, 1], mybir.dt.int32, isOutput=False)
    y = nc.declare_dram_parameter(
        "y", [SEQ_PER_CORE * MAX_SL, F], mybir.dt.float32, isOutput=True
    )

    src_off = [0]
    for bj in bases:
        src_off.append(src_off[-1] + bj)

    with ExitStack() as ctx:
        stage = ctx.enter_context(nc.sbuf_tensor([n_parts, RU * F], mybir.dt.float32))
        dst_t = ctx.enter_context(nc.sbuf_tensor([n_parts, 1], mybir.dt.int32))
        sem_in = ctx.enter_context(nc.semaphore("sem_in"))
        sem_big = ctx.enter_context(nc.semaphore("sem_big"))
        sem_scat = ctx.enter_context(nc.semaphore("sem_scat"))
        block = ctx.enter_context(nc.Block())

        @block.scalar
        def _(scalar):
            # second HWDGE ring: remainder table + stage, then 2 base copies
            scalar.dma_start(out=dst_t[:, :], in_=dst[:, :]).then_inc(sem_in, 16)
            xr = x[n_base : n_base + rem_rows, :].rearrange(
                "(p q) f -> p (q f)", p=n_parts
            )
            scalar.dma_start(out=stage[:, :], in_=xr).then_inc(sem_in, 16)
            for j in (1, 3):
                scalar.dma_start(
                    out=y[j * MAX_SL : j * MAX_SL + bases[j], :],
                    in_=x[src_off[j] : src_off[j] + bases[j], :],
                ).then_inc(sem_big, 16)

        @block.sync
        def _(sync):
            for j in (0, 2):
                sync.dma_start(
                    out=y[j * MAX_SL : j * MAX_SL + bases[j], :],
                    in_=x[src_off[j] : src_off[j] + bases[j], :],
                ).then_inc(sem_big, 16)
            sync.wait_ge(sem_big, 64)

        @block.gpsimd
        def _(gp):
            gp.wait_ge(sem_in, 32)
            yv = y.rearrange("(n e) f -> n (e f)", e=RU)
            gp.indirect_dma_start(
                out=yv[:, :],
                out_offset=bass.IndirectOffsetOnAxis(ap=dst_t[:, 0:1], axis=0),
                in_=stage[:, :],
                in_offset=None,
            ).then_inc(sem_scat, 16)
            gp.wait_ge(sem_scat, 16)
    return nc


def _groups():
    return [[k, 15 - k, 16 + k, 31 - k] for k in range(NCORES)]


def _host_fallback(S, L, max_sl):
    out = np.zeros((len(L), max_sl, S.shape[1]), dtype=S.dtype)
    off = 0
    for b, ln in enumerate(L):
        out[b, :ln] = S[off : off + ln]
        off += ln
    return out


def _fast_path_ok(S, L, max_sl):
    if (
        max_sl != MAX_SL
        or len(L) != B
        or S.shape[1] != F
        or int(L.sum()) != S.shape[0]
        or np.any(L % 64)
        or np.any(L < 64)
        or np.any(L > max_sl)
    ):
        return False
    groups = _groups()
    totals = [sum(int(L[s]) for s in g) for g in groups]
    if len(set(totals)) != 1:
        return False
    bases = [min(int(L[g[j]]) for g in groups) for j in range(SEQ_PER_CORE)]
    rem = totals[0] - sum(bases)
    n_parts = rem // RU
    if rem % RU or n_parts < 2 or n_parts > 128:
        return False
    return True


def _prepare(S, L):
    offsets = np.zeros(B + 1, dtype=np.int64)
    np.cumsum(L, out=offsets[1:])
    groups = _groups()
    bases = [min(int(L[g[j]]) for g in groups) for j in range(SEQ_PER_CORE)]
    rem_rows = sum(int(L[s]) for s in groups[0]) - sum(bases)
    n_parts = rem_rows // RU

    in_maps = []
    for k in range(NCORES):
        xs = []
        tails = []
        dst_k = np.zeros((n_parts, 1), dtype=np.int32)
        p = 0
        for j, s in enumerate(groups[k]):
            ln = int(L[s])
            bj = bases[j]
            xs.append(S[offsets[s] : offsets[s] + bj])
            tails.append(S[offsets[s] + bj : offsets[s] + ln])
            for u in range((ln - bj) // RU):
                dst_k[p, 0] = (j * MAX_SL + bj) // RU + u
                p += 1
        assert p == n_parts
        x_k = np.concatenate(xs + tails, axis=0)
        in_maps.append({"x": x_k, "dst": dst_k})

    key = (tuple(bases), rem_rows)
    if key not in _NC_CACHE:
        _NC_CACHE[key] = _build_nc(*key)
    return _NC_CACHE[key], in_maps, groups


def _assemble(results, groups):
    out = np.empty((B, MAX_SL, F), dtype=np.float32)
    for k in range(NCORES):
        yk = np.asarray(results[k]["y"]).reshape(SEQ_PER_CORE, MAX_SL, F)
        for j, s in enumerate(groups[k]):
            out[s] = yk[j]
    return out


def kernel(concatenated_sequences, sequence_lengths, max_sl):
    S = np.ascontiguousarray(np.asarray(concatenated_sequences, dtype=np.float32))
    L = np.asarray(sequence_lengths).reshape(-1).astype(np.int64)
    max_sl = int(np.asarray(max_sl))

    if not _fast_path_ok(S, L, max_sl):
        return _host_fallback(S, L, max_sl)

    nc, in_maps, groups = _prepare(S, L)
    res = run_bass_kernel_spmd(nc, in_maps, list(range(NCORES))).results
    return _assemble(res, groups)
